# revision 1
# baseline (speedup 1.0000x reference)
"""AttentionBlock (GroupNorm + 8-head attention + proj + residual) for
Trainium2, data-parallel over batch across 8 NeuronCores (2 batches/core).

Structure (per batch):
  h   = GroupNorm(x)          -> fp8e4 h8
  q,k = W h + b               -> fp8 in the (d%32, d//32) split layout
                                 (fp8 DoubleRow matmuls, W scaled x64)
  v^T                         -> fp8 (produced transposed; h8 stationary)
  per head pair:
    S^T = k^T q               -> fp8 DoubleRow (32-part tiles), both heads
                                 packed into one [128,1024] psum
    E   = exp(S * scale)      -> fp8e4 straight out of ScalarE, or a
                                 Schraudolph int8 fast-exp on DVE (~46% of
                                 tiles) to split the softmax-exp load
    AV  = v @ E               -> fp8 DoubleRow, ones-col accumulates the
                                 softmax denominator in-psum
    h'  = AV * (8/denom)      -> fp8 (denom rows ScalarE-copied, broadcast
                                 via a 0.125 K=1 bf16 matmul, one exact DVE
                                 reciprocal, multiply)
  out = W_p h' / 512 + b_eff + x   (fp8 DoubleRow + K=1 bf16 bias matmul)

fp8 scaling: weights stored as 64*W (keeps N(0,0.02) weights out of e4m3
denormals), h' stored as 8*h'; compensated exactly by 1/64 on the q/k/v
copies and 1/512 on the proj output.
"""

import numpy as np

import concourse.bass as bass
import concourse.tile as tile
from concourse import mybir
from concourse.bass_utils import run_bass_kernel_spmd

F32 = mybir.dt.float32
BF16 = mybir.dt.bfloat16
F8E4 = mybir.dt.float8e4
I8 = mybir.dt.int8
AF = mybir.ActivationFunctionType
ALU = mybir.AluOpType
DR = mybir.MatmulPerfMode.DoubleRow

N_CORES = 8
B, C, H, W = 16, 512, 32, 32
HW = H * W            # 1024
NH, HD = 8, 64
GROUPS = 32
GS = C // GROUPS      # 16 channels per group
EPS = 1e-5
BPC = B // N_CORES    # 2 batches per core
CT = C // 128         # 4 channel tiles
JT = HW // 128        # 8 spatial tiles (attention j)
JTP = JT // 2         # 4 j-tile pairs (DoubleRow AV)
NSL = HW // 512       # 2 moving-dim slices of 512
NPAIR = NH // 2       # 4 head pairs
SCALE = HD ** -0.5
WS = 64.0             # fp8 weight prescale
HS = 8.0              # fp8 h' prescale (folded into the r broadcast)
RS = WS * HS          # proj psum overall scale (512)
LOG2E = 1.4426950408889634
# Schraudolph fp8e4m3 exp: bits = trunc(8*log2e*x + SCH_C); SCH_C tuned for
# min softmax error (56 = e4m3 exponent bias<<3, +0.5 trunc->round, -0.46
# Schraudolph shift)
SCH_C = 56.0 - 0.46
# exp-tile engine routing: A=ScalarE exact exp, D=DVE / P=Pool Schraudolph
EXP_PATTERN = "ADADAADADADAADADADAADADADAADADAD"


def _split_multi_waits(nc):
    """walrus's per-instruction sync-wait slots are limited (LDWEIGHTS and
    DMA DIRECT2D reject >1). Move excess waits onto a preceding NoOp on the
    same engine — the NX sequencer processes waits in stream order, so the
    semantics are unchanged."""
    n_split = 0
    for f in nc.m.functions:
        for bb in f.blocks:
            out = []
            for inst in bb.instructions:
                si = inst.sync_info
                if si is not None and si.on_wait and len(si.on_wait) > 1:
                    waits = list(si.on_wait)
                    evsem_ok = inst.engine in (
                        mybir.EngineType.PE, mybir.EngineType.SP
                    )
                    for w in waits[:-1]:
                        if evsem_ok:
                            carrier = mybir.InstEventSemaphore(
                                name=nc.get_next_instruction_name()
                            )
                        else:
                            carrier = mybir.InstDrain(
                                name=nc.get_next_instruction_name()
                            )
                        carrier.engine = inst.engine
                        carrier.debug = inst.debug
                        carrier.sync_info = mybir.SyncInfo(
                            on_wait=[w], on_update=[]
                        )
                        out.append(carrier)
                        n_split += 1
                    si.on_wait = waits[-1:]
                    inst.sync_info = si
                out.append(inst)
            bb.instructions[:] = out
    return n_split


def build_nc(split_waits=True, has_qk_bias=False, has_beff=False):
    nc = bass.Bass()
    x_in = nc.declare_dram_parameter("x_local", [BPC, C, HW], F32, isOutput=False)
    wqkvT = nc.declare_dram_parameter("w_qkvT", [C, 3 * C], F32, isOutput=False)
    wprojT = nc.declare_dram_parameter("w_projT", [C, C], F32, isOutput=False)
    bq_d = nc.declare_dram_parameter("b_q", [C], F32, isOutput=False)
    bk_d = nc.declare_dram_parameter("b_k", [C], F32, isOutput=False)
    beff_d = nc.declare_dram_parameter("b_eff512", [C], F32, isOutput=False)
    gam_d = nc.declare_dram_parameter("gn_gamma", [C], F32, isOutput=False)
    bet_d = nc.declare_dram_parameter("gn_beta", [C], F32, isOutput=False)
    ind_d = nc.declare_dram_parameter("gn_ind", [128, GROUPS // CT], F32, isOutput=False)
    rep_d = nc.declare_dram_parameter("gn_rep", [GROUPS // CT, 128], F32, isOutput=False)
    out_d = nc.declare_dram_parameter("out_local", [BPC, C, HW], F32, isOutput=True)

    with tile.TileContext(nc) as tc:
        with (
            tc.tile_pool(name="wpool", bufs=1) as wpool,
            tc.tile_pool(name="cpool", bufs=1) as cpool,
            tc.tile_pool(name="hpool", bufs=2) as hpool,
            tc.tile_pool(name="qkpool", bufs=2) as qkpool,
            tc.tile_pool(name="vhpool", bufs=1) as vhpool,
            tc.tile_pool(name="epool", bufs=5) as epool,
            tc.tile_pool(name="spool", bufs=4) as spool,
            tc.tile_pool(name="npool", bufs=4) as npool,
            tc.tile_pool(name="opool", bufs=3) as opool,
            tc.tile_pool(name="ps2", bufs=2, space="PSUM") as ps2,
            tc.tile_pool(name="pssp", bufs=2, space="PSUM") as pssp,
            tc.tile_pool(name="psav", bufs=1, space="PSUM") as psav,
        ):
            # x for batch 0 first: GroupNorm is the head of the critical
            # path; the weight loads only gate qkv ~10us later
            xl_tiles = []
            h8_tiles = []
            for b in range(BPC):
                xl_tiles.append(hpool.tile([128, CT, HW], F32, tag="xl",
                                           name=f"xl{b}"))
                h8_tiles.append(hpool.tile([128, CT, HW], F8E4, tag="h8",
                                           name=f"h8{b}"))
            # ---------- constants (small DMAs first: GroupNorm needs
            # ind16/rep/gamma/beta immediately; weights only gate qkv) ----------
            ind16 = cpool.tile([128, GROUPS // CT], F32, tag="ind16")
            nc.sync.dma_start(out=ind16, in_=ind_d.ap())
            rep_sb = cpool.tile([GROUPS // CT, 128], F32, tag="rep")
            nc.sync.dma_start(out=rep_sb, in_=rep_d.ap())
            for kt in range(CT):
                dma_eng = nc.scalar if kt % 2 == 1 else nc.sync
                dma_eng.dma_start(
                    out=xl_tiles[0][:, kt, :], in_=x_in[0, kt * 128:(kt + 1) * 128, :]
                )
            bq_sb = cpool.tile([128, CT], F32, tag="bq")
            bk_sb = cpool.tile([128, CT], F32, tag="bk")
            gam_sb = cpool.tile([128, CT], F32, tag="gam")
            bet_sb = cpool.tile([128, CT], F32, tag="bet")
            for sb, d in ((gam_sb, gam_d), (bet_sb, bet_d)):
                nc.sync.dma_start(out=sb, in_=d.rearrange("(m p) -> p m", p=128))
            ones_bf = cpool.tile([1, 512], BF16, tag="onesbf")
            nc.vector.memset(ones_bf, 1.0)
            # per-partition 1/RS for the proj output rescale
            rsc_sb = cpool.tile([128, 1], F32, tag="rsc")
            nc.vector.memset(rsc_sb, 1.0 / RS)
            eps_sb = cpool.tile([128, 1], F32, tag="eps")
            nc.vector.memset(eps_sb, EPS)

            # ---------- weights (loaded once) ----------
            # fp8 weights at 64x (keeps 0.02-scale weights out of denormals).
            # wq rides SP right after gamma/beta; wk takes the ACT queue
            # behind batch 0's x so q AND k are ready ~5us earlier; wp (only
            # needed by proj ~70us in) goes after batch 1's x; the biases
            # (unused in the zero-bias fast path) load last.
            wq8 = wpool.tile([128, CT, C], F8E4, tag="wq8")
            wk8 = wpool.tile([128, CT, C], F8E4, tag="wk8")
            wv8 = wpool.tile([128, CT, C], F8E4, tag="wv8")
            wp8 = wpool.tile([128, CT, C], F8E4, tag="wp8")
            w_srcs = (
                (wq8, wqkvT[:, 0:C], nc.sync, nc.vector),
                (wk8, wqkvT[:, C:2 * C], nc.scalar, nc.gpsimd),
                (wv8, wqkvT[:, 2 * C:3 * C], nc.sync, nc.vector),
                (wp8, wprojT[:, :], nc.sync, nc.gpsimd),
            )
            for wi, (w_sb, w_src, dq, eng) in enumerate(w_srcs):
                if wi == 3:
                    for kt in range(CT):
                        nc.sync.dma_start(
                            out=xl_tiles[1][:, kt, :],
                            in_=x_in[1, kt * 128:(kt + 1) * 128, :]
                        )
                wtmp = wpool.tile([128, CT, C], F32, tag="wtmp", bufs=2,
                                  name=f"wtmp{wi}")
                dq.dma_start(
                    out=wtmp, in_=w_src.rearrange("(kt p) o -> p kt o", p=128)
                )
                eng.tensor_scalar(out=w_sb, in0=wtmp, scalar1=WS, scalar2=None,
                                  op0=ALU.mult)
            for sb, d in ((bq_sb, bq_d), (bk_sb, bk_d)):
                nc.sync.dma_start(out=sb, in_=d.rearrange("(m p) -> p m", p=128))
            # proj bias row (512*b_eff), bf16, on partition 0 for the K=1
            # psum-init matmul
            befftmp = cpool.tile([1, C], F32, tag="befftmp")
            nc.sync.dma_start(out=befftmp, in_=beff_d.rearrange("(a c) -> a c", a=1))
            beff_bf = cpool.tile([1, C], BF16, tag="beffbf")
            nc.vector.tensor_copy(beff_bf, befftmp)

            # AV stationary ones/zeros columns are batch-invariant: write once.
            # Even heads ("A"): v in cols 0-63, ones col 64 -> denominator on
            # psum row 64. Odd heads ("B"): v in cols 64-127 (lane-aligned
            # with final destination), ones col 32 -> denominator on row 32.
            # merged AV stationary: cols 0-63 v_a, col 64 ones (A denom on
            # psum row 64); cols 65-192 are the B stationary: B-col 32 (=97)
            # ones -> B denom on psum row 32, B-cols 64-127 (=129-192) v_b
            # dual-fp8 LDWEIGHTS needs even column offsets/widths: A block
            # cols 0:66 (64 v + ones col 64 + zero pad), B block at 66:194
            # (ones at 98 -> B denom row 32, v_b at 130:194 -> rows 64-127)
            vh_t = vhpool.tile([128, JTP, 2, NPAIR, 196], F8E4, tag="vh")
            nc.vector.memset(vh_t[:, :, :, :, 64:65], 1.0)
            nc.gpsimd.memset(vh_t[:, :, :, :, 65:132], 0.0)
            nc.gpsimd.memset(vh_t[:, :, :, :, 100:101], 1.0)
            # denominator-broadcast selector: K=1 row of 0.125 (recip of
            # denom/8 bakes the fp8 h' prescale of 8 into r)
            sel_bf = cpool.tile([1, 64], BF16, tag="selbf")
            nc.vector.memset(sel_bf, 1.0 / HS)


            # ---------- phase closures ----------
            # Engine instruction streams execute in program order, so batch
            # phases are hand-interleaved below: batch 1's GroupNorm/qkv are
            # emitted in the middle of batch 0's attention units, keeping
            # every engine fed across the batch boundary.
            q_tiles, k_tiles, hav_tiles = {}, {}, {}
            exp_ctr = [0]

            def gn_phase(b):
                xl_t, h8_t = xl_tiles[b], h8_tiles[b]
                for kt in range(CT):
                    st = spool.tile([128, 2, 6], F32, tag="bnst")
                    for s in range(2):
                        nc.vector.bn_stats(
                            out=st[:, s, :], in_=xl_t[:, kt, s * 512:(s + 1) * 512]
                        )
                    s3 = spool.tile([128, 3], F32, tag="s3")
                    nc.vector.bn_aggr(out=s3[:, 0:2], in_=st)
                    nc.vector.tensor_mul(s3[:, 2:3], s3[:, 0:1], s3[:, 0:1])
                    gps = ps2.tile([128, 512], F32, tag="ps2t")
                    nc.tensor.matmul(
                        gps[0:8, 0:3], lhsT=ind16, rhs=s3, start=True, stop=True
                    )
                    g3 = spool.tile([8, 3], F32, tag="g3")
                    nc.vector.tensor_copy(g3, gps[0:8, 0:3])
                    g2 = spool.tile([8, 2], F32, tag="g2")
                    nc.vector.tensor_copy(g2[:, 0:1], g3[:, 0:1])
                    vg = spool.tile([8, 2], F32, tag="vg")
                    nc.vector.tensor_add(vg[:, 0:1], g3[:, 1:2], g3[:, 2:3])
                    nc.vector.tensor_mul(vg[:, 1:2], g3[:, 0:1], g3[:, 0:1])
                    nc.vector.tensor_sub(vg[:, 0:1], vg[:, 0:1], vg[:, 1:2])
                    # rstd = exp(-0.5*ln(var+eps)): stays in the
                    # natural_log_exp ACT table set
                    nc.scalar.activation(
                        out=vg[:, 1:2], in_=vg[:, 0:1], func=AF.Ln,
                        bias=eps_sb[0:8, :], scale=1.0,
                    )
                    nc.scalar.activation(
                        out=g2[:, 1:2], in_=vg[:, 1:2], func=AF.Exp,
                        scale=-0.5,
                    )
                    bc = ps2.tile([128, 512], F32, tag="ps2t")
                    nc.tensor.matmul(
                        bc[0:128, 0:2], lhsT=rep_sb, rhs=g2, start=True, stop=True
                    )
                    ab = spool.tile([128, 3], F32, tag="ab")
                    nc.vector.tensor_mul(ab[:, 0:1], bc[:, 1:2], gam_sb[:, kt:kt + 1])
                    nc.vector.tensor_mul(ab[:, 2:3], bc[:, 0:1], ab[:, 0:1])
                    nc.vector.tensor_sub(ab[:, 1:2], bet_sb[:, kt:kt + 1], ab[:, 2:3])
                    nc.gpsimd.tensor_scalar(
                        out=h8_t[:, kt, :], in0=xl_t[:, kt, :],
                        scalar1=ab[:, 0:1], scalar2=ab[:, 1:2],
                        op0=ALU.mult, op1=ALU.add,
                    )

            def qk_phase(b, ms):
                # q8/k8 layout for the DoubleRow S matmul: head h lives on
                # partitions 32*(h%4)..+32 of group g=h//4, with d split as
                # (d%32 -> partition, d//32 -> free dim). The m-th psum tile
                # holds (g=m//2, d_hi=m%2) via host-side weight column
                # permutation.
                if b not in q_tiles:
                    q_tiles[b] = qkpool.tile([128, 2, 2, HW], F8E4, tag="q",
                                             name=f"q{b}")
                    k_tiles[b] = qkpool.tile([128, 2, 2, HW], F8E4, tag="k",
                                             name=f"k{b}")
                h8_t = h8_tiles[b]
                for m in ms:
                    for w_sb, b_sb, dst in (
                        (wq8, bq_sb, q_tiles[b]), (wk8, bk_sb, k_tiles[b]),
                    ):
                        for isl in range(NSL):
                            pq = ps2.tile([128, 512], F32, tag="ps2t")
                            for tp in range(CT // 2):
                                nc.tensor.matmul(
                                    pq[:, :],
                                    lhsT=w_sb[:, 2 * tp:2 * tp + 2,
                                              m * 128:(m + 1) * 128],
                                    rhs=h8_t[:, 2 * tp:2 * tp + 2,
                                             isl * 512:(isl + 1) * 512],
                                    start=(tp == 0), stop=(tp == CT // 2 - 1),
                                    perf_mode=DR,
                                )
                            if has_qk_bias:
                                nc.vector.tensor_scalar(
                                    out=dst[:, m // 2, m % 2,
                                            isl * 512:(isl + 1) * 512],
                                    in0=pq[:, :],
                                    scalar1=1.0 / WS, scalar2=b_sb[:, m:m + 1],
                                    op0=ALU.mult, op1=ALU.add,
                                )
                            else:
                                nc.scalar.activation(
                                    out=dst[:, m // 2, m % 2,
                                            isl * 512:(isl + 1) * 512],
                                    in_=pq[:, :], func=AF.Copy,
                                    scale=1.0 / WS,
                                )

            def v_phase(b, mjs):
                # v, produced transposed ([j, o]) with h8 as the stationary
                h8_t = h8_tiles[b]
                for mj in mjs:
                    pv = ps2.tile([128, 512], F32, tag="ps2t")
                    for tp in range(CT // 2):
                        nc.tensor.matmul(
                            pv[:, 0:512],
                            lhsT=h8_t[:, 2 * tp:2 * tp + 2,
                                      mj * 128:(mj + 1) * 128],
                            rhs=wv8[:, 2 * tp:2 * tp + 2, :],
                            start=(tp == 0), stop=(tp == CT // 2 - 1),
                            perf_mode=DR,
                        )
                    pv_h = pv[:, 0:512].rearrange(
                        "p (hp a d) -> p hp a d", hp=NPAIR, a=2
                    )
                    nc.scalar.activation(
                        out=vh_t[:, mj // 2, mj % 2, :, 0:64],
                        in_=pv_h[:, :, 0, :], func=AF.Copy, scale=1.0 / WS,
                    )
                    nc.scalar.activation(
                        out=vh_t[:, mj // 2, mj % 2, :, 132:196],
                        in_=pv_h[:, :, 1, :], func=AF.Copy, scale=1.0 / WS,
                    )

            def att_core(b, hp, isl):
                # one (head pair, i-half): 1-bank PSUM tiles throughout; the
                # S-psum ring runs 4 deep and the AV accumulators
                # double-buffer, so consecutive units pipeline instead of
                # ping-ponging PE against the exp engines
                if b not in hav_tiles:
                    # attention output, fp8 at 8x (the 8 comes from the
                    # denominator broadcast at denom/8)
                    hav_tiles[b] = hpool.tile([128, NPAIR, HW], F8E4,
                                              tag="h8av", name=f"h8av{b}")
                q_t, k_t, h8av = q_tiles[b], k_tiles[b], hav_tiles[b]
                sl = slice(isl * 512, (isl + 1) * 512)
                avA = psav.tile([68, 512], F32, tag="avA")
                avB = psav.tile([128, 512], F32, tag="avB")
                for jbp in range(JTP):
                    # both heads' E in one tile: A in cols 0:512, B 512:1024
                    e8 = epool.tile([128, 2, 1024], F8E4, tag="e8")
                    for js in range(2):
                        jb = 2 * jbp + js
                        # both heads' S^T packed in one 2-bank psum so one
                        # exp op covers the pair
                        pss = pssp.tile([128, 1024], F32, tag="pss")
                        for a in range(2):
                            h = 2 * hp + a
                            g, base = h // 4, 32 * (h % 4)
                            nc.tensor.matmul(
                                pss[:, a * 512:(a + 1) * 512],
                                lhsT=k_t[base:base + 32, g, :,
                                         jb * 128:(jb + 1) * 128],
                                rhs=q_t[base:base + 32, g, :, sl],
                                start=True, stop=True,
                                perf_mode=DR,
                                tile_position=(base, 0),
                            )
                        eng = EXP_PATTERN[exp_ctr[0] % len(EXP_PATTERN)]
                        exp_ctr[0] += 1
                        if eng == "A":
                            nc.scalar.activation(
                                out=e8[:, js, :], in_=pss[:, :],
                                func=AF.Exp, scale=SCALE,
                            )
                        else:
                            # Schraudolph fast exp straight into fp8e4 bits:
                            # round(8*log2e*scale*S + SCH_C) on DVE (GPSIMD
                            # cannot read PSUM)
                            nc.vector.tensor_scalar(
                                out=e8.bitcast(I8)[:, js, :],
                                in0=pss[:, :],
                                scalar1=8.0 * LOG2E * SCALE,
                                scalar2=SCH_C,
                                op0=ALU.mult, op1=ALU.add,
                            )
                    # AV accumulation (unnormalized, DoubleRow over j-tile
                    # pairs); ones columns accumulate softmax denominators
                    # on avA row 64 / avB row 32
                    nc.tensor.matmul(
                        avA[:, :], lhsT=vh_t[:, jbp, :, hp, 0:68],
                        rhs=e8[:, :, 0:512],
                        start=(jbp == 0), stop=(jbp == JTP - 1),
                        perf_mode=DR,
                    )
                    nc.tensor.matmul(
                        avB[:, :], lhsT=vh_t[:, jbp, :, hp, 68:196],
                        rhs=e8[:, :, 512:1024],
                        start=(jbp == 0), stop=(jbp == JTP - 1),
                        perf_mode=DR,
                    )
                return avA, avB

            def att_norm(b, hp, isl, avA, avB):
                # normalize: h' = 8*av/denom. Denominator rows copied to bf16
                # on ScalarE, broadcast to all partitions at denom/8 via a
                # 0.125-valued K=1 bf16 matmul (plain mode: DoubleRow can't
                # target dst partition 64), one exact DVE reciprocal, then a
                # multiply. Emitted one unit late (software pipelining).
                h8av = hav_tiles[b]
                sl = slice(isl * 512, (isl + 1) * 512)
                dbfA = npool.tile([1, 512], BF16, tag="dbfA")
                dbfB = npool.tile([1, 512], BF16, tag="dbfB")
                nc.scalar.activation(out=dbfA, in_=avA[64:65, :], func=AF.Copy)
                nc.vector.tensor_copy(dbfB, avB[32:33, :])
                Dp = ps2.tile([128, 512], F32, tag="ps2t")
                nc.tensor.matmul(
                    Dp[0:64, :], lhsT=sel_bf, rhs=dbfA,
                    start=True, stop=True,
                )
                nc.tensor.matmul(
                    Dp[64:128, :], lhsT=sel_bf, rhs=dbfB,
                    start=True, stop=True,
                )
                rcp = npool.tile([128, 512], F32, tag="rcp")
                nc.vector.reciprocal(out=rcp, in_=Dp[:, :])
                nc.vector.tensor_mul(h8av[0:64, hp, sl], avA[0:64, :],
                                     rcp[0:64, :])
                nc.vector.tensor_mul(h8av[64:128, hp, sl], avB[64:128, :],
                                     rcp[64:128, :])

            def proj_phase(b, ms):
                xl_t, h8av = xl_tiles[b], hav_tiles[b]
                for m in ms:
                    ot = opool.tile([128, HW], F32, tag="ot")
                    for isl in range(NSL):
                        sl = slice(isl * 512, (isl + 1) * 512)
                        po = ps2.tile([128, 512], F32, tag="ps2t")
                        if has_beff:
                            # general path: bias row seeds the psum (mixing
                            # bf16 into the fp8-DR group)
                            nc.tensor.matmul(
                                po[:, :],
                                lhsT=beff_bf[:, m * 128:(m + 1) * 128],
                                rhs=ones_bf[:, :],
                                start=True, stop=False,
                            )
                        for tp in range(CT // 2):
                            nc.tensor.matmul(
                                po[:, :],
                                lhsT=wp8[:, 2 * tp:2 * tp + 2,
                                         m * 128:(m + 1) * 128],
                                rhs=h8av[:, 2 * tp:2 * tp + 2, sl],
                                start=(tp == 0 and not has_beff),
                                stop=(tp == CT // 2 - 1),
                                perf_mode=DR,
                            )
                        nc.vector.scalar_tensor_tensor(
                            out=ot[:, sl], in0=po[:, :], scalar=rsc_sb,
                            in1=xl_t[:, m, sl], op0=ALU.mult, op1=ALU.add,
                        )
                    dma_eng = nc.scalar if (b == 1 and m % 2 == 1) else nc.sync
                    dma_eng.dma_start(
                        out=out_d[b, m * 128:(m + 1) * 128, :], in_=ot
                    )

            # ---------- hand-interleaved, software-pipelined schedule ----
            # units u = (b, hp, isl) run core(u_n) ... norm(u_{n-1}) so the
            # PE stream never waits for a denominator; batch 1's GN/qkv are
            # spread through batch 0's attention to bridge the transition.
            units = [(b, hp, isl) for b in range(BPC)
                     for hp in range(NPAIR) for isl in range(NSL)]
            # extra work to emit after core(u_i), for i in 0..15
            extras = {
                0: lambda: qk_phase(0, (2, 3)),
                2: lambda: gn_phase(1),
                4: lambda: qk_phase(1, (0, 1)),
                5: lambda: qk_phase(1, (2, 3)),
                # vh is single-buffered: batch 1's v MUST follow the last
                # batch-0 attention core (program order = dependency order)
                7: lambda: v_phase(1, range(JT)),
                8: lambda: proj_phase(0, (0, 1)),
                9: lambda: proj_phase(0, (2, 3)),
            }
            gn_phase(0)
            qk_phase(0, (0, 1))
            v_phase(0, range(JT))
            pending = None
            for i, (b, hp, isl) in enumerate(units):
                av = att_core(b, hp, isl)
                if pending is not None:
                    att_norm(*pending)
                pending = (b, hp, isl, *av)
                if i in extras:
                    extras[i]()
            att_norm(*pending)
            proj_phase(1, (0, 1, 2, 3))
    if split_waits:
        _split_multi_waits(nc)
    return nc


_NC_CACHE = {}


def _get_nc(has_qk_bias=False, has_beff=False):
    key = ("nc", has_qk_bias, has_beff)
    if key not in _NC_CACHE:
        _NC_CACHE[key] = build_nc(has_qk_bias=has_qk_bias, has_beff=has_beff)
    return _NC_CACHE[key]


def make_in_maps(x, gn_gamma, gn_beta, w_qkv, b_qkv, w_proj, b_proj):
    f = np.float32
    x = np.ascontiguousarray(np.asarray(x, dtype=f)).reshape(B, C, HW)
    w_qkvT = np.ascontiguousarray(np.asarray(w_qkv, dtype=f).T)
    w_projT = np.ascontiguousarray(np.asarray(w_proj, dtype=f).T)
    b_qkv = np.asarray(b_qkv, dtype=f)
    # q/k output-channel permutation for the DoubleRow S layout: psum tile
    # X=(g, d_hi), partition p=(h%4)*32 + d%32 holds channel
    # c=(4g + p//32)*64 + d_hi*32 + p%32
    perm = np.empty(C, dtype=np.int64)
    for X in range(CT):
        g, d_hi = X // 2, X % 2
        for p in range(128):
            perm[X * 128 + p] = (4 * g + p // 32) * 64 + d_hi * 32 + p % 32
    w_qkvT = np.concatenate([
        w_qkvT[:, 0:C][:, perm],
        w_qkvT[:, C:2 * C][:, perm],
        w_qkvT[:, 2 * C:3 * C],
    ], axis=1)
    w_qkvT = np.ascontiguousarray(w_qkvT)
    b_q = np.ascontiguousarray(b_qkv[0:C][perm])
    b_k = np.ascontiguousarray(b_qkv[C:2 * C][perm])
    b_v = b_qkv[2 * C:3 * C]
    # softmax rows sum to 1, so v's bias passes straight through attention:
    # fold it into the projection bias. Stored at 512x (the proj psum scale).
    b_eff512 = np.ascontiguousarray(
        RS * (np.asarray(w_proj, dtype=f) @ b_v + np.asarray(b_proj, dtype=f))
    )
    gn_gamma = np.ascontiguousarray(np.asarray(gn_gamma, dtype=f))
    gn_beta = np.ascontiguousarray(np.asarray(gn_beta, dtype=f))
    n_gpt = GROUPS // CT   # groups per 128-channel tile
    gn_ind = np.zeros((128, n_gpt), dtype=f)
    gn_rep = np.zeros((n_gpt, 128), dtype=f)
    for g in range(n_gpt):
        gn_ind[g * GS:(g + 1) * GS, g] = 1.0 / GS
        gn_rep[g, g * GS:(g + 1) * GS] = 1.0
    in_maps = []
    for c in range(N_CORES):
        in_maps.append({
            "x_local": np.ascontiguousarray(x[c * BPC:(c + 1) * BPC]),
            "w_qkvT": w_qkvT,
            "w_projT": w_projT,
            "b_q": b_q,
            "b_k": b_k,
            "b_eff512": b_eff512,
            "gn_gamma": gn_gamma,
            "gn_beta": gn_beta,
            "gn_ind": gn_ind,
            "gn_rep": gn_rep,
        })
    return in_maps


def kernel(x, gn_gamma, gn_beta, w_qkv, b_qkv, w_proj, b_proj):
    b_qkv_a = np.asarray(b_qkv)
    has_qk_bias = bool(np.any(b_qkv_a[0:2 * C]))
    has_beff = bool(np.any(b_qkv_a[2 * C:])) or bool(np.any(np.asarray(b_proj)))
    nc = _get_nc(has_qk_bias, has_beff)
    in_maps = make_in_maps(x, gn_gamma, gn_beta, w_qkv, b_qkv, w_proj, b_proj)
    res = run_bass_kernel_spmd(nc, in_maps, list(range(N_CORES)))
    out = np.empty((B, C, HW), dtype=np.float32)
    for c in range(N_CORES):
        out[c * BPC:(c + 1) * BPC] = res.results[c]["out_local"]
    return out.reshape(B, C, H, W)

